# revision 10
# baseline (speedup 1.0000x reference)
"""Trainium2 Bass kernel for a single-head attention block (B=4, S=2048, D=1024).

reference:
    x = gelu(tokens); q,k,v = x@W{q,k,v} + b; scores = q@k^T/sqrt(D)
    out = softmax(scores)@v @ Wo + bo + tokens

Sharding: 8 cores = 4 batches x 2 query-halves. Each core computes K/V for its
full batch (keys permuted so its own query rows come first — softmax over keys
is permutation invariant) and attention for its 1024 query rows.

Per-core device pipeline (bf16 matmuls, fp32 accumulation):
  xT = gelu(tokensT)                 # ACT, [d,s] layout, queries first
  kT[e,s] = (Wk^T-stationary)x       # lhsT=Wk tile, rhs=xT  -> +bk (per-part)
  qT[e,s] likewise
  v[s,d]  = (xT-stationary)Wv        # lhsT=xT tile, rhs=Wv  -> +bv (bcast)
  scoresT[sk,sq] = k q^T             # lhsT=kT, rhs=qT
  expT = exp(scoresT/32)             # ACT, no max-subtraction (|s|<3 checked)
  S[sq] = ones^T expT                # rank-1 matmuls, PE
  mixedU[sq,d] = expT^T v            # lhsT=expT, rhs=v; * 1/S per-partition
  mixedT = PE-transpose(mixedN)
  out[sq,e] = mixedT^T Wo + (tokens_q + bo)   # residual folded host-side
"""

import numpy as np
import ml_dtypes

B, S, D = 4, 2048, 1024
NCORES = 8
SQ = S // 2          # query rows per core
P = 128
DT = 8               # d / 128
ST = S // P          # 16 seq tiles
N512 = 512

_COMPILED = {}


def _build_program():
    from contextlib import ExitStack

    import concourse.bass as bass
    import concourse.tile as tile
    from concourse import bacc, mybir
    from concourse.masks import make_identity

    f32 = mybir.dt.float32
    bf16 = mybir.dt.bfloat16
    AF = mybir.ActivationFunctionType

    nc = bacc.Bacc("TRN2", target_bir_lowering=False, debug=False,
                   num_devices=NCORES)

    tokT = nc.dram_tensor("tokT", [D, S], bf16, kind="ExternalInput")
    resid = nc.dram_tensor("resid", [SQ, D], f32, kind="ExternalInput")
    wq = nc.dram_tensor("wq", [D, D], bf16, kind="ExternalInput")
    wk = nc.dram_tensor("wk", [D, D], bf16, kind="ExternalInput")
    wv = nc.dram_tensor("wv", [D, D], bf16, kind="ExternalInput")
    wo = nc.dram_tensor("wo", [D, D], bf16, kind="ExternalInput")
    bq_d = nc.dram_tensor("bq", [D], f32, kind="ExternalInput")
    bk_d = nc.dram_tensor("bk", [D], f32, kind="ExternalInput")
    bv_d = nc.dram_tensor("bv", [D], f32, kind="ExternalInput")
    bo_d = nc.dram_tensor("bo", [D], f32, kind="ExternalInput")
    out_d = nc.dram_tensor("out", [SQ, D], f32, kind="ExternalOutput")

    ts = bass.ts

    with tile.TileContext(nc) as tc, ExitStack() as ctx:
        pers = ctx.enter_context(tc.tile_pool(name="pers", bufs=1))
        kT = pers.tile([P, DT, S], bf16, tag="kT")
        qT = pers.tile([P, DT, SQ], bf16, tag="qT")
        v = pers.tile([P, ST, D], bf16, tag="v")
        ones = pers.tile([P, 1], bf16, tag="ones")
        ident = pers.tile([P, P], bf16, tag="ident")
        bqk = pers.tile([P, 2, DT], f32, tag="bqk")  # [:,0,:]=bq  [:,1,:]=bk

        nc.vector.memset(ones, 1.0)
        make_identity(nc, ident)
        nc.sync.dma_start(bqk[:, 0, :], bq_d.ap().rearrange("(t p) -> p t", p=P))
        nc.sync.dma_start(bqk[:, 1, :], bk_d.ap().rearrange("(t p) -> p t", p=P))

        psum = ctx.enter_context(tc.tile_pool(name="psum", bufs=3, space="PSUM"))
        psum_s = ctx.enter_context(tc.tile_pool(name="psum_s", bufs=2, space="PSUM"))
        psum_t = ctx.enter_context(tc.tile_pool(name="psum_t", bufs=2, space="PSUM"))

        # ---------------- phase 1: gelu + projections ----------------
        with ExitStack() as ph1:
            wpool = ph1.enter_context(tc.tile_pool(name="w1", bufs=1))
            wq_sb = wpool.tile([P, DT, D], bf16, tag="wq")
            wk_sb = wpool.tile([P, DT, D], bf16, tag="wk")
            wv_sb = wpool.tile([P, DT, D], bf16, tag="wv")
            bv_sb = wpool.tile([P, D], f32, tag="bv")
            xpool = ph1.enter_context(tc.tile_pool(name="xp", bufs=1))
            xTl = [xpool.tile([P, S], bf16, tag=f"xT{t}", name=f"xT{t}")
                   for t in range(DT)]
            stag = ph1.enter_context(tc.tile_pool(name="stag", bufs=2))

            nc.sync.dma_start(wk_sb, wk.ap().rearrange("(t p) e -> p t e", p=P))
            nc.sync.dma_start(wq_sb, wq.ap().rearrange("(t p) e -> p t e", p=P))
            nc.sync.dma_start(wv_sb, wv.ap().rearrange("(t p) e -> p t e", p=P))
            nc.gpsimd.dma_start(
                bv_sb, bass.AP(tensor=bv_d, offset=0, ap=[[0, P], [1, D]]))

            for t in range(DT):
                stg = stag.tile([P, S], bf16, tag="tok")
                nc.sync.dma_start(stg, tokT.ap()[ts(t, P), :])
                nc.scalar.activation(xTl[t], stg, AF.Gelu)

            # kT / qT : lhsT = W-slice, rhs = xT
            for te in range(DT):
                for c in range(S // N512):
                    ps = psum.tile([P, N512], f32, tag="mm")
                    for td in range(DT):
                        nc.tensor.matmul(ps, wk_sb[:, td, ts(te, P)],
                                         xTl[td][:, ts(c, N512)],
                                         start=(td == 0), stop=(td == DT - 1))
                    nc.vector.tensor_scalar_add(kT[:, te, ts(c, N512)], ps,
                                                bqk[:, 1, te:te + 1])
                for c in range(SQ // N512):
                    ps = psum.tile([P, N512], f32, tag="mm")
                    for td in range(DT):
                        nc.tensor.matmul(ps, wq_sb[:, td, ts(te, P)],
                                         xTl[td][:, ts(c, N512)],
                                         start=(td == 0), stop=(td == DT - 1))
                    nc.vector.tensor_scalar_add(qT[:, te, ts(c, N512)], ps,
                                                bqk[:, 0, te:te + 1])
            # v : lhsT = xT-slice, rhs = Wv
            for tsq in range(ST):
                for dc in range(D // N512):
                    ps = psum.tile([P, N512], f32, tag="mm")
                    for td in range(DT):
                        nc.tensor.matmul(ps, xTl[td][:, ts(tsq, P)],
                                         wv_sb[:, td, ts(dc, N512)],
                                         start=(td == 0), stop=(td == DT - 1))
                    nc.vector.tensor_add(v[:, tsq, ts(dc, N512)], ps,
                                         bv_sb[:, ts(dc, N512)])

        # ---------------- phase 2: attention + out-proj ----------------
        with ExitStack() as ph2:
            w2pool = ph2.enter_context(tc.tile_pool(name="w2", bufs=1))
            wo_sb = w2pool.tile([P, DT, D], bf16, tag="wo")
            bo_sb = w2pool.tile([P, D], f32, tag="bo")
            nc.sync.dma_start(wo_sb, wo.ap().rearrange("(t p) e -> p t e", p=P))
            nc.gpsimd.dma_start(
                bo_sb, bass.AP(tensor=bo_d, offset=0, ap=[[0, P], [1, D]]))
            epool = ph2.enter_context(tc.tile_pool(name="ep", bufs=2))
            work = ph2.enter_context(tc.tile_pool(name="wk2", bufs=2))

            for c in range(SQ // N512):          # sq chunks of 512
                expT = epool.tile([P, ST, N512], bf16, tag="expT")
                for tk in range(ST):
                    ps = psum.tile([P, N512], f32, tag="mm")
                    for te in range(DT):
                        nc.tensor.matmul(ps, kT[:, te, ts(tk, P)],
                                         qT[:, te, ts(c, N512)],
                                         start=(te == 0), stop=(te == DT - 1))
                    nc.scalar.activation(expT[:, tk, :], ps, AF.Exp,
                                         scale=1.0 / 32.0)
                rS = work.tile([P, 4], f32, tag="rS")
                for sl in range(4):              # 128-row q slices in chunk
                    psS = psum_s.tile([P, 1], f32, tag="S")
                    for tk in range(ST):
                        nc.tensor.matmul(psS, expT[:, tk, ts(sl, P)], ones,
                                         start=(tk == 0), stop=(tk == ST - 1))
                    nc.vector.reciprocal(rS[:, sl:sl + 1], psS)
                for sl in range(4):
                    mixedN = work.tile([P, D], bf16, tag="mixedN")
                    for dc in range(D // N512):
                        ps = psum.tile([P, N512], f32, tag="mm")
                        for tk in range(ST):
                            nc.tensor.matmul(ps, expT[:, tk, ts(sl, P)],
                                             v[:, tk, ts(dc, N512)],
                                             start=(tk == 0), stop=(tk == ST - 1))
                        nc.vector.tensor_scalar_mul(mixedN[:, ts(dc, N512)], ps,
                                                    rS[:, sl:sl + 1])
                    mixT = work.tile([P, DT, P], bf16, tag="mixT")
                    for td in range(DT):
                        pst = psum_t.tile([P, P], bf16, tag="tr")
                        nc.tensor.transpose(pst, mixedN[:, ts(td, P)], ident)
                        nc.vector.tensor_copy(mixT[:, td, :], pst)
                    row = (c * 4 + sl) * P
                    res_sb = work.tile([P, D], f32, tag="res")
                    nc.sync.dma_start(res_sb, resid.ap()[row:row + P, :])
                    nc.vector.tensor_add(res_sb, res_sb, bo_sb)
                    out_sb = work.tile([P, D], f32, tag="osb")
                    for ec in range(D // N512):
                        ps = psum.tile([P, N512], f32, tag="mm")
                        for td in range(DT):
                            nc.tensor.matmul(ps, mixT[:, td, :],
                                             wo_sb[:, td, ts(ec, N512)],
                                             start=(td == 0), stop=(td == DT - 1))
                        nc.vector.tensor_add(out_sb[:, ts(ec, N512)], ps,
                                             res_sb[:, ts(ec, N512)])
                    nc.sync.dma_start(out_d.ap()[row:row + P, :], out_sb)

    nc.compile()
    return nc


def _get_program():
    if "nc" not in _COMPILED:
        _COMPILED["nc"] = _build_program()
    return _COMPILED["nc"]


def make_in_maps(tokens, Wq, bq, Wk, bk, Wv, bv, Wo, bo):
    tokens = np.asarray(tokens, dtype=np.float32)
    bf = ml_dtypes.bfloat16
    wq_b = np.ascontiguousarray(np.asarray(Wq, np.float32).astype(bf))
    wk_b = np.ascontiguousarray(np.asarray(Wk, np.float32).astype(bf))
    wv_b = np.ascontiguousarray(np.asarray(Wv, np.float32).astype(bf))
    wo_b = np.ascontiguousarray(np.asarray(Wo, np.float32).astype(bf))
    bq = np.asarray(bq, np.float32)
    bk = np.asarray(bk, np.float32)
    bv = np.asarray(bv, np.float32)
    bo = np.asarray(bo, np.float32)

    in_maps = []
    for c in range(NCORES):
        b, h = divmod(c, 2)
        q_rows = tokens[b, h * SQ:(h + 1) * SQ]
        o_rows = tokens[b, (1 - h) * SQ:(2 - h) * SQ]
        perm = np.concatenate([q_rows, o_rows], axis=0)      # [S, D]
        in_maps.append({
            "tokT": np.ascontiguousarray(perm.T.astype(bf)),  # [D, S] bf16
            "resid": np.ascontiguousarray(q_rows),            # [SQ, D] f32
            "wq": wq_b, "wk": wk_b, "wv": wv_b, "wo": wo_b,
            "bq": bq, "bk": bk, "bv": bv, "bo": bo,
        })
    return in_maps


def gather_out(results):
    out = np.empty((B, S, D), np.float32)
    for c in range(NCORES):
        b, h = divmod(c, 2)
        out[b, h * SQ:(h + 1) * SQ] = results[c]["out"]
    return out


def kernel(tokens, Wq, bq, Wk, bk, Wv, bv, Wo, bo):
    from concourse.bass_utils import run_bass_kernel_spmd

    in_maps = make_in_maps(tokens, Wq, bq, Wk, bk, Wv, bv, Wo, bo)
    nc = _get_program()
    res = run_bass_kernel_spmd(nc, in_maps, core_ids=list(range(NCORES)),
                               trace=False)
    return gather_out(res.results)


# revision 16
# speedup vs baseline: 1.1960x; 1.1960x over previous
"""Trainium2 Bass kernel for a single-head attention block (B=4, S=2048, D=1024).

reference:
    x = gelu(tokens); q,k,v = x@W{q,k,v} + b; scores = q@k^T/sqrt(D)
    out = softmax(scores)@v @ Wo + bo + tokens

Sharding: 8 cores = 4 batches x 2 query-halves. Core c=2b+h handles batch b,
query rows [h*1024, (h+1)*1024). Each core projects K/V only for its OWN rows;
the halves are exchanged pairwise via AllGather (DRAM bounce buffers), giving
both cores the full K/V in global row order.

Per-core device pipeline (bf16 matmuls, fp32 accumulation):
  warm-up MMs                        # keep PE HAM at K=8/8 from t=0
  xT = gelu(tokT)                    # ACT, [d, s_own]
  kTown[e,s_own] = (Wk^T-stat)x     -> DRAM bounce -> AllGather -> kT[e,s]
  vown[s_own,d]  = (xT-stat)Wv      -> DRAM bounce -> AllGather -> v[s,d]
  qT[e,s_own] likewise (local only)
  scoresT[sk,sq] = k q^T             # lhsT=kT, rhs=qT
  expT = exp(scoresT/32)             # ACT, no max-subtraction (|s|<3 checked)
  S[sq] = ones^T expT                # rank-1 matmuls on PE
  mixedU[sq,d] = expT^T v            # lhsT=expT, rhs=v; * 1/S per-partition
  mixedT = PE-transpose(mixedN)
  out[sq,e] = mixedT^T Wo + (tokens_q + bo)
"""

import numpy as np
import ml_dtypes

B, S, D = 4, 2048, 1024
NCORES = 8
SQ = S // 2          # query rows per core (own rows)
P = 128
DT = 8               # d / 128
ST = S // P          # 16 seq tiles
SQT = SQ // P        # 8 own seq tiles
N512 = 512
WARMUP_MMS = 48

_COMPILED = {}


def _build_program():
    from contextlib import ExitStack

    import concourse.bass as bass
    import concourse.tile as tile
    from concourse import bacc, mybir
    from concourse.masks import make_identity

    f32 = mybir.dt.float32
    bf16 = mybir.dt.bfloat16
    AF = mybir.ActivationFunctionType

    nc = bacc.Bacc("TRN2", target_bir_lowering=False, debug=False,
                   num_devices=NCORES)

    tokT = nc.dram_tensor("tokT", [D, SQ], bf16, kind="ExternalInput")
    resid = nc.dram_tensor("resid", [SQ, D], f32, kind="ExternalInput")
    wq = nc.dram_tensor("wq", [D, D], bf16, kind="ExternalInput")
    wk = nc.dram_tensor("wk", [D, D], bf16, kind="ExternalInput")
    wv = nc.dram_tensor("wv", [D, D], bf16, kind="ExternalInput")
    wo = nc.dram_tensor("wo", [D, D], bf16, kind="ExternalInput")
    bq_d = nc.dram_tensor("bq", [D], f32, kind="ExternalInput")
    bk_d = nc.dram_tensor("bk", [D], f32, kind="ExternalInput")
    bv_d = nc.dram_tensor("bv", [D], f32, kind="ExternalInput")
    bo_d = nc.dram_tensor("bo", [D], f32, kind="ExternalInput")
    out_d = nc.dram_tensor("out", [SQ, D], f32, kind="ExternalOutput")

    ts = bass.ts
    groups = [[2 * i, 2 * i + 1] for i in range(NCORES // 2)]

    with tile.TileContext(nc) as tc, ExitStack() as ctx:
        pers = ctx.enter_context(tc.tile_pool(name="pers", bufs=1))
        kT = pers.tile([P, DT, S], bf16, tag="kT")
        qT = pers.tile([P, DT, SQ], bf16, tag="qT")
        v = pers.tile([P, ST, D], bf16, tag="v")
        ones = pers.tile([P, 1], bf16, tag="ones")
        ident = pers.tile([P, P], bf16, tag="ident")
        bqk = pers.tile([P, 2, DT], f32, tag="bqk")  # [:,0,:]=bq  [:,1,:]=bk
        wscr = pers.tile([P, P], bf16, tag="wscr")
        wsink = pers.tile([P, P], f32, tag="wsink")

        dram = ctx.enter_context(tc.tile_pool(name="dram", bufs=1, space="DRAM"))
        kb_in = dram.tile([D, SQ], bf16, tag="kb_in")    # kTown, [e, s_own]
        kb_out = dram.tile([2, D, SQ], bf16, tag="kb_out")
        vb_in = dram.tile([SQ, D], bf16, tag="vb_in")    # vown, [s_own, d]
        vb_out = dram.tile([2, SQ, D], bf16, tag="vb_out")

        psum = ctx.enter_context(tc.tile_pool(name="psum", bufs=3, space="PSUM"))
        psum_s = ctx.enter_context(tc.tile_pool(name="psum_s", bufs=2, space="PSUM"))
        psum_t = ctx.enter_context(tc.tile_pool(name="psum_t", bufs=2, space="PSUM"))
        psum_w = ctx.enter_context(tc.tile_pool(name="psum_w", bufs=1, space="PSUM"))

        # --- PE warm-up: dense trivial matmuls so HAM hits K=8/8 before the
        # real stream begins (gelu+DMA head would otherwise leave PE cold).
        nc.vector.memset(wscr, 0.0)
        wps = psum_w.tile([P, P], f32, tag="warm")
        for i in range(WARMUP_MMS):
            nc.tensor.matmul(wps, wscr, wscr, start=(i == 0),
                             stop=(i == WARMUP_MMS - 1))
        nc.vector.tensor_copy(wsink, wps)

        nc.vector.memset(ones, 1.0)
        make_identity(nc, ident)
        nc.sync.dma_start(bqk[:, 0, :], bq_d.ap().rearrange("(t p) -> p t", p=P))
        nc.sync.dma_start(bqk[:, 1, :], bk_d.ap().rearrange("(t p) -> p t", p=P))

        # ---------------- phase 1: gelu + projections + exchange ------------
        with ExitStack() as ph1:
            wpool = ph1.enter_context(tc.tile_pool(name="w1", bufs=1))
            wq_sb = wpool.tile([P, DT, D], bf16, tag="wq")
            wk_sb = wpool.tile([P, DT, D], bf16, tag="wk")
            wv_sb = wpool.tile([P, DT, D], bf16, tag="wv")
            bv_sb = wpool.tile([P, D], f32, tag="bv")
            kTo = wpool.tile([P, DT, SQ], bf16, tag="kTo")
            vo = wpool.tile([P, SQT, D], bf16, tag="vo")
            xT = ph1.enter_context(tc.tile_pool(name="xp", bufs=1)).tile(
                [P, DT, SQ], bf16, tag="xT")
            stag = ph1.enter_context(tc.tile_pool(name="stag", bufs=2))

            nc.sync.dma_start(wk_sb, wk.ap().rearrange("(t p) e -> p t e", p=P))
            nc.sync.dma_start(wv_sb, wv.ap().rearrange("(t p) e -> p t e", p=P))
            nc.sync.dma_start(wq_sb, wq.ap().rearrange("(t p) e -> p t e", p=P))
            nc.gpsimd.dma_start(
                bv_sb, bass.AP(tensor=bv_d, offset=0, ap=[[0, P], [1, D]]))

            for t in range(DT):
                stg = stag.tile([P, SQ], bf16, tag="tok")
                nc.sync.dma_start(stg, tokT.ap()[ts(t, P), :])
                nc.scalar.activation(xT[:, t, :], stg, AF.Gelu)

            # kT own half: lhsT = Wk-slice, rhs = xT
            for te in range(DT):
                for c in range(SQ // N512):
                    ps = psum.tile([P, N512], f32, tag="mm")
                    for td in range(DT):
                        nc.tensor.matmul(ps, wk_sb[:, td, ts(te, P)],
                                         xT[:, td, ts(c, N512)],
                                         start=(td == 0), stop=(td == DT - 1))
                    nc.vector.tensor_scalar_add(kTo[:, te, ts(c, N512)], ps,
                                                bqk[:, 1, te:te + 1])
            # exchange kT halves (bounce holds [e, s_own] row-major)
            nc.sync.dma_start(kb_in[:].rearrange("(t p) s -> p t s", p=P), kTo)
            nc.gpsimd.collective_compute(
                "AllGather", mybir.AluOpType.bypass, replica_groups=groups,
                ins=[kb_in[:].opt()], outs=[kb_out[:].opt()])
            for r in range(2):
                nc.sync.dma_start(
                    kT[:, :, r * SQ:(r + 1) * SQ],
                    kb_out[r].rearrange("(t p) s -> p t s", p=P))

            # v own half: lhsT = xT-slice, rhs = Wv
            for tsq in range(SQT):
                for dc in range(D // N512):
                    ps = psum.tile([P, N512], f32, tag="mm")
                    for td in range(DT):
                        nc.tensor.matmul(ps, xT[:, td, ts(tsq, P)],
                                         wv_sb[:, td, ts(dc, N512)],
                                         start=(td == 0), stop=(td == DT - 1))
                    nc.vector.tensor_add(vo[:, tsq, ts(dc, N512)], ps,
                                         bv_sb[:, ts(dc, N512)])
            nc.sync.dma_start(vb_in[:].rearrange("(t p) d -> p t d", p=P), vo)
            nc.gpsimd.collective_compute(
                "AllGather", mybir.AluOpType.bypass, replica_groups=groups,
                ins=[vb_in[:].opt()], outs=[vb_out[:].opt()])
            for r in range(2):
                nc.sync.dma_start(
                    v[:, r * SQT:(r + 1) * SQT, :],
                    vb_out[r].rearrange("(t p) d -> p t d", p=P))

            # qT (local only)
            for te in range(DT):
                for c in range(SQ // N512):
                    ps = psum.tile([P, N512], f32, tag="mm")
                    for td in range(DT):
                        nc.tensor.matmul(ps, wq_sb[:, td, ts(te, P)],
                                         xT[:, td, ts(c, N512)],
                                         start=(td == 0), stop=(td == DT - 1))
                    nc.vector.tensor_scalar_add(qT[:, te, ts(c, N512)], ps,
                                                bqk[:, 0, te:te + 1])

        # ---------------- phase 2: attention + out-proj ----------------
        with ExitStack() as ph2:
            w2pool = ph2.enter_context(tc.tile_pool(name="w2", bufs=1))
            wo_sb = w2pool.tile([P, DT, D], bf16, tag="wo")
            bo_sb = w2pool.tile([P, D], f32, tag="bo")
            nc.sync.dma_start(wo_sb, wo.ap().rearrange("(t p) e -> p t e", p=P))
            nc.gpsimd.dma_start(
                bo_sb, bass.AP(tensor=bo_d, offset=0, ap=[[0, P], [1, D]]))
            epool = ph2.enter_context(tc.tile_pool(name="ep", bufs=2))
            work = ph2.enter_context(tc.tile_pool(name="wk2", bufs=2))

            for c in range(SQ // N512):          # sq chunks of 512
                expT = epool.tile([P, ST, N512], bf16, tag="expT")
                for tk in range(ST):
                    ps = psum.tile([P, N512], f32, tag="mm")
                    for te in range(DT):
                        nc.tensor.matmul(ps, kT[:, te, ts(tk, P)],
                                         qT[:, te, ts(c, N512)],
                                         start=(te == 0), stop=(te == DT - 1))
                    nc.scalar.activation(expT[:, tk, :], ps, AF.Exp,
                                         scale=1.0 / 32.0)
                rS = work.tile([P, 4], f32, tag="rS")
                for sl in range(4):              # 128-row q slices in chunk
                    psS = psum_s.tile([P, 1], f32, tag="S")
                    for tk in range(ST):
                        nc.tensor.matmul(psS, expT[:, tk, ts(sl, P)], ones,
                                         start=(tk == 0), stop=(tk == ST - 1))
                    nc.vector.reciprocal(rS[:, sl:sl + 1], psS)
                for sl in range(4):
                    mixedN = work.tile([P, D], bf16, tag="mixedN")
                    for dc in range(D // N512):
                        ps = psum.tile([P, N512], f32, tag="mm")
                        for tk in range(ST):
                            nc.tensor.matmul(ps, expT[:, tk, ts(sl, P)],
                                             v[:, tk, ts(dc, N512)],
                                             start=(tk == 0), stop=(tk == ST - 1))
                        nc.vector.tensor_scalar_mul(mixedN[:, ts(dc, N512)], ps,
                                                    rS[:, sl:sl + 1])
                    mixT = work.tile([P, DT, P], bf16, tag="mixT")
                    for td in range(DT):
                        pst = psum_t.tile([P, P], bf16, tag="tr")
                        nc.tensor.transpose(pst, mixedN[:, ts(td, P)], ident)
                        nc.vector.tensor_copy(mixT[:, td, :], pst)
                    row = (c * 4 + sl) * P
                    res_sb = work.tile([P, D], f32, tag="res")
                    nc.sync.dma_start(res_sb, resid.ap()[row:row + P, :])
                    nc.gpsimd.tensor_add(res_sb, res_sb, bo_sb)
                    out_sb = work.tile([P, D], f32, tag="osb")
                    for ec in range(D // N512):
                        ps = psum.tile([P, N512], f32, tag="mm")
                        for td in range(DT):
                            nc.tensor.matmul(ps, mixT[:, td, :],
                                             wo_sb[:, td, ts(ec, N512)],
                                             start=(td == 0), stop=(td == DT - 1))
                        nc.vector.tensor_add(out_sb[:, ts(ec, N512)], ps,
                                             res_sb[:, ts(ec, N512)])
                    nc.sync.dma_start(out_d.ap()[row:row + P, :], out_sb)

    nc.compile()
    return nc


def _get_program():
    if "nc" not in _COMPILED:
        _COMPILED["nc"] = _build_program()
    return _COMPILED["nc"]


def make_in_maps(tokens, Wq, bq, Wk, bk, Wv, bv, Wo, bo):
    tokens = np.asarray(tokens, dtype=np.float32)
    bf = ml_dtypes.bfloat16
    wq_b = np.ascontiguousarray(np.asarray(Wq, np.float32).astype(bf))
    wk_b = np.ascontiguousarray(np.asarray(Wk, np.float32).astype(bf))
    wv_b = np.ascontiguousarray(np.asarray(Wv, np.float32).astype(bf))
    wo_b = np.ascontiguousarray(np.asarray(Wo, np.float32).astype(bf))
    bq = np.asarray(bq, np.float32)
    bk = np.asarray(bk, np.float32)
    bv = np.asarray(bv, np.float32)
    bo = np.asarray(bo, np.float32)

    in_maps = []
    for c in range(NCORES):
        b, h = divmod(c, 2)
        q_rows = tokens[b, h * SQ:(h + 1) * SQ]
        in_maps.append({
            "tokT": np.ascontiguousarray(q_rows.T.astype(bf)),  # [D, SQ] bf16
            "resid": np.ascontiguousarray(q_rows),              # [SQ, D] f32
            "wq": wq_b, "wk": wk_b, "wv": wv_b, "wo": wo_b,
            "bq": bq, "bk": bk, "bv": bv, "bo": bo,
        })
    return in_maps


def gather_out(results):
    out = np.empty((B, S, D), np.float32)
    for c in range(NCORES):
        b, h = divmod(c, 2)
        out[b, h * SQ:(h + 1) * SQ] = results[c]["out"]
    return out


def kernel(tokens, Wq, bq, Wk, bk, Wv, bv, Wo, bo):
    from concourse.bass_utils import run_bass_kernel_spmd

    in_maps = make_in_maps(tokens, Wq, bq, Wk, bk, Wv, bv, Wo, bo)
    nc = _get_program()
    res = run_bass_kernel_spmd(nc, in_maps, core_ids=list(range(NCORES)),
                               trace=False)
    return gather_out(res.results)
